# revision 1
# baseline (speedup 1.0000x reference)
"""TRN2 Bass kernel for nn_DerivNet2D.

Reference computation (per sample x in R^2):
    h1 = W1 @ x + b1;  z1 = tanh(h1)            (1024)
    h2 = W2 @ z1 + b2; z2 = tanh(h2)            (512)
    y  = W3 @ z2 + b3                           (1)
    dy/dx_k = W3 @ (dz2 * (W2 @ (dz1 * W1[:,k])))   k = 1, 2
    returns (y, v1, v2) = (y, dy/dx2, -dy/dx1)

Strategy:
  * Pure data parallel: x is split into 8 shards of 8192 samples, one per
    NeuronCore; weights are replicated.  SPMD module via run_bass_kernel_spmd.
  * On-chip layout is feature-major: activations are [features, nx_tile] so
    the 1024-dim contraction sits on partitions.
  * The two input derivatives use ONE reverse-mode backward pass instead of
    two forward-mode chains:
        A = dz2 * w3;  B = W2.T @ A;  dy/dx_k = sum_i W1[i,k]*dz1[i,n]*B[i,n]
    turning 3 big [512x1024] matmul chains into 2.
  * Mixed matmul precision chosen from an error model:
      - forward H2 = W2 @ z1 runs in bf16 (weight load overlaps -> 216ns/MM),
      - everything else (L1, backward, y, dydx) runs in float32r (full fp32
        storage, 1 cycle/row, ~1.5e-4 relative) because the derivative
        outputs are first-order sensitive to backward-operand rounding.
      - dz1 is computed from the f32-precision z1, not the bf16 copy.
  * Small matmuls are packed onto the PE array: L1 (K=2) runs 4 row-groups
    concurrently; y (M=1) runs in column-group 32 concurrently with the
    dydx matmul (M=2) in column-group 0.
  * x arrives [nx, 2] (sample-major); the k-on-partitions transpose is done
    on-chip with PE transposes of 4 sub-blocks.  This permutes the sample
    order within the shard; the host undoes it with a reshape.
"""

import numpy as np
from contextlib import ExitStack

import concourse.bacc as bacc
import concourse.mybir as mybir
import concourse.tile as tile
from concourse.bass import ds, ts
from concourse.masks import make_identity

F32 = mybir.dt.float32
F32R = mybir.dt.float32r
BF16 = mybir.dt.bfloat16
AF = mybir.ActivationFunctionType
ALU = mybir.AluOpType

NCORES = 8
NX = 65536
NXL = NX // NCORES      # 8192 samples per core
NT = 512                # samples per tile
TILES = NXL // NT       # 16
JB = 4                  # x-transpose sub-blocks
TSUB = 16               # t-values per sub-block; JB * TSUB * 128 == NXL

PACK_L1 = True          # L1 K=2 matmuls in 4 concurrent PE row-groups
PACK_Y = False          # col-group-32 y matmul fails walrus ISA check

_CACHE = {}


def build():
    nc = bacc.Bacc(None, target_bir_lowering=False)
    x = nc.dram_tensor("x", [NXL, 2], F32, kind="ExternalInput")
    W1 = nc.dram_tensor("W1", [1024, 2], F32, kind="ExternalInput")
    b1 = nc.dram_tensor("b1", [1024], F32, kind="ExternalInput")
    W2 = nc.dram_tensor("W2", [512, 1024], F32, kind="ExternalInput")
    b2 = nc.dram_tensor("b2", [512], F32, kind="ExternalInput")
    W3 = nc.dram_tensor("W3", [1, 512], F32, kind="ExternalInput")
    b3 = nc.dram_tensor("b3", [1], F32, kind="ExternalInput")
    out = nc.dram_tensor("out", [3, NXL], F32, kind="ExternalOutput")

    with ExitStack() as ctx:
        tc = ctx.enter_context(tile.TileContext(nc))
        sg = ctx.enter_context(tc.tile_pool(name="sg", bufs=1))
        pz1 = ctx.enter_context(tc.tile_pool(name="pz1", bufs=2))
        pdz1 = ctx.enter_context(tc.tile_pool(name="pdz1", bufs=2))
        pz2 = ctx.enter_context(tc.tile_pool(name="pz2", bufs=2))
        pA = ctx.enter_context(tc.tile_pool(name="pA", bufs=2))
        pC = ctx.enter_context(tc.tile_pool(name="pC", bufs=1))
        pyv = ctx.enter_context(tc.tile_pool(name="pyv", bufs=2))
        ph1 = ctx.enter_context(tc.tile_pool(name="ph1", bufs=2, space="PSUM"))
        ph2 = ctx.enter_context(tc.tile_pool(name="ph2", bufs=2, space="PSUM"))
        pB = ctx.enter_context(tc.tile_pool(name="pB", bufs=2, space="PSUM"))
        psm = ctx.enter_context(tc.tile_pool(name="psm", bufs=2, space="PSUM"))

        # ---- preload / preprocess ------------------------------------
        ident = sg.tile([128, 128], F32)
        make_identity(nc, ident)

        # x_sb[p, j, 2t+k] = x[j*2048 + p*16 + t, k]
        x_sb = sg.tile([128, JB, 2 * TSUB], F32)
        nc.sync.dma_start(
            out=x_sb,
            in_=x[:, :].rearrange("(j p t) k -> p j (t k)", j=JB, p=128, t=TSUB),
        )

        # W1T replicated at partition bases {0,32,64,96} for row-group packing
        n_g = 4 if PACK_L1 else 1
        W1T4 = sg.tile([(n_g - 1) * 32 + 2, 1024], F32R)
        nc.sync.dma_start(
            out=W1T4[0:2, :], in_=W1[:, :].rearrange("m k -> k m").bitcast(F32R)
        )
        for g in range(1, n_g):
            nc.sync.dma_start(out=W1T4[32 * g : 32 * g + 2, :], in_=W1T4[0:2, :])

        # W1c[p, i, :] = [W1[i*128+p, 1], W1[i*128+p, 0]]  (flipped so row0 of
        # the dydx matmul output is dy/dx2 = v1)
        W1c = sg.tile([128, 8, 2], F32R)
        nc.sync.dma_start(
            out=W1c[:, :, 0:1],
            in_=W1[:, 1:2].rearrange("(c p) k -> p c k", p=128).bitcast(F32R),
        )
        nc.sync.dma_start(
            out=W1c[:, :, 1:2],
            in_=W1[:, 0:1].rearrange("(c p) k -> p c k", p=128).bitcast(F32R),
        )

        b1s = sg.tile([128, 8], F32)
        nc.sync.dma_start(out=b1s, in_=b1[:].rearrange("(c p) -> p c", p=128))
        b2s = sg.tile([128, 4], F32)
        nc.sync.dma_start(out=b2s, in_=b2[:].rearrange("(c p) -> p c", p=128))
        # b3 at partition 32 (where the packed y row lives)
        b3s = sg.tile([33, 1], F32)
        nc.sync.dma_start(out=b3s[0:1, :], in_=b3[:].unsqueeze(0))
        nc.sync.dma_start(out=b3s[32:33, :], in_=b3[:].unsqueeze(0))

        w3s = sg.tile([128, 4], F32)
        nc.sync.dma_start(out=w3s, in_=W3[0, :].rearrange("(c p) -> p c", p=128))
        w3r = sg.tile([128, 4], F32R)
        nc.sync.dma_start(
            out=w3r, in_=W3[0, :].rearrange("(c p) -> p c", p=128).bitcast(F32R)
        )
        w3n = sg.tile([128, 4], F32)
        nc.vector.tensor_scalar_mul(w3n, w3s, -1.0)

        # sflip = [+1, -1] per partition: 1 - 2*partition_idx via iota
        sfi = sg.tile([2, 1], mybir.dt.int32)
        nc.gpsimd.iota(sfi, pattern=[[0, 1]], base=0, channel_multiplier=1)
        sflip = sg.tile([2, 1], F32)
        nc.vector.tensor_scalar(
            out=sflip, in0=sfi, scalar1=-2.0, scalar2=1.0, op0=ALU.mult, op1=ALU.add
        )

        # W2 natural blocks (f32r): lhsT of the backward matmul B = W2.T @ A
        W2n = sg.tile([128, 4, 1024], F32R)
        for c in range(4):
            nc.sync.dma_start(out=W2n[:, c, :], in_=W2[ts(c, 128), :].bitcast(F32R))

        # W2T (f32r): lhsT of the forward H2 = W2 @ Z1, via PE block transposes
        W2T = sg.tile([128, 8, 512], F32R)
        for c in range(4):
            for i in range(8):
                pt = psm.tile([128, 128], F32, tag="sm", name="pt")
                nc.tensor.transpose(
                    pt, W2n[:, c, ds(i * 128, 128)].bitcast(F32), ident
                )
                nc.vector.tensor_copy(W2T[:, i, ds(c * 128, 128)], pt)

        # XT4[k + 32g, t*512 + j*128 + p] = x[j*2048 + p*16 + t, k], g=0..n_g-1
        XT_big = sg.tile([2 * TSUB, JB, 128], F32R)
        for j in range(JB):
            pxt = psm.tile([2 * TSUB, 128], F32, tag="sm", name="pxt")
            nc.tensor.transpose(pxt, x_sb[:, j, :], ident)
            nc.vector.tensor_copy(XT_big[:, j, :], pxt)
        XT4 = sg.tile([(n_g - 1) * 32 + 2, NXL], F32R)
        for t in range(TSUB):
            for k in range(2):
                q = 2 * t + k
                nc.sync.dma_start(
                    out=XT4[k : k + 1, ds(t * NT, NT)], in_=XT_big[q : q + 1, :, :]
                )
        for g in range(1, n_g):
            nc.sync.dma_start(out=XT4[32 * g : 32 * g + 2, :], in_=XT4[0:2, :])

        # ---- main loop over nx tiles ---------------------------------
        for T in range(TILES):
            sl = ds(T * NT, NT)

            # L1: h1 = W1 @ xT; z1 = tanh(h1 + b1)  [n_g concurrent row-groups]
            z1r = pz1.tile([128, 8, NT], F32R, tag="z1r", name="z1r")
            dz1 = pdz1.tile([128, 8, NT], F32, tag="dz1", name="dz1")
            for c1 in range(8):
                g = c1 % n_g
                p1 = ph1.tile([128, NT], F32, tag="h1", name="p1")
                nc.tensor.matmul(
                    p1,
                    W1T4[32 * g : 32 * g + 2, ts(c1, 128)],
                    XT4[32 * g : 32 * g + 2, sl],
                    start=True, stop=True,
                    tile_position=(32 * g, 0) if PACK_L1 else None,
                )
                nc.scalar.activation(
                    z1r[:, c1, :], p1, AF.Tanh, bias=b1s[:, c1 : c1 + 1]
                )
                nc.scalar.activation(
                    dz1[:, c1, :], z1r[:, c1, :].bitcast(F32), AF.Square
                )
                nc.vector.tensor_scalar(
                    out=dz1[:, c1, :], in0=dz1[:, c1, :],
                    scalar1=-1.0, scalar2=1.0, op0=ALU.mult, op1=ALU.add,
                )

            # L2 fwd (bf16): h2 = W2 @ z1; z2 = tanh(h2 + b2)
            z2 = pz2.tile([128, 4, NT], F32R, tag="z2", name="z2")
            for c in range(4):
                p2 = ph2.tile([128, NT], F32, tag="h2", name="p2")
                for j in range(8):
                    nc.tensor.matmul(
                        p2, W2T[:, j, ds(c * 128, 128)], z1r[:, j, :],
                        start=(j == 0), stop=(j == 7),
                    )
                nc.scalar.activation(
                    z2[:, c, :], p2, AF.Tanh, bias=b2s[:, c : c + 1]
                )

            # A = w3 * (1 - z2^2): per-chunk square + w3 scalars
            A = pA.tile([128, 4, NT], F32R, tag="A", name="A")
            for c in range(4):
                nc.scalar.activation(
                    A[:, c, :], z2[:, c, :].bitcast(F32), AF.Square
                )
                nc.vector.tensor_scalar(
                    out=A[:, c, :], in0=A[:, c, :].bitcast(F32),
                    scalar1=w3n[:, c : c + 1], scalar2=w3s[:, c : c + 1],
                    op0=ALU.mult, op1=ALU.add,
                )

            # y = W3 @ z2 + b3
            pyy = psm.tile([1, NT], F32, tag="sm", name="pyy")
            for c in range(4):
                nc.tensor.matmul(
                    pyy[0:1, :], w3r[:, c : c + 1], z2[:, c, :],
                    start=(c == 0), stop=(c == 3),
                )
            ytile = pyv.tile([1, NT], F32, tag="yt", name="ytile")
            nc.scalar.add(ytile[0:1, :], pyy[0:1, :], b3s[0:1, 0:1])
            nc.sync.dma_start(out=out[0:1, sl], in_=ytile[0:1, :])

            # backward: B = W2.T @ A;  C = B * dz1
            C = pC.tile([128, 8, NT], F32R, tag="C", name="C")
            for i in range(8):
                pb = pB.tile([128, NT], F32, tag="B", name="pb")
                for c in range(4):
                    nc.tensor.matmul(
                        pb, W2n[:, c, ds(i * 128, 128)], A[:, c, :],
                        start=(c == 0), stop=(c == 3),
                    )
                nc.vector.tensor_mul(C[:, i, :], pb, dz1[:, i, :])

            # y (col-group 32) runs concurrently with dydx (col-group 0)
            pyd = psm.tile([2, NT], F32, tag="sm", name="pyd")
            for i in range(8):
                nc.tensor.matmul(
                    pyd[0:2, :], W1c[:, i, :], C[:, i, :],
                    start=(i == 0), stop=(i == 7),
                )
            vtile = pyv.tile([2, NT], F32, tag="vt", name="vtile")
            nc.vector.tensor_scalar_mul(vtile[0:2, :], pyd[0:2, :], sflip[0:2, 0:1])
            nc.sync.dma_start(out=out[1:3, sl], in_=vtile[0:2, :])

    nc.compile()
    return nc


def _unpermute(o):
    """Undo the on-chip sample permutation: column v = t*512 + j*128 + p of the
    device output holds sample n = j*2048 + p*16 + t of the shard."""
    return np.ascontiguousarray(
        o.reshape(3, TILES, JB, 128).transpose(0, 2, 3, 1).reshape(3, NXL)
    )


def kernel(x, W1, b1, W2, b2, W3, b3):
    from concourse.bass_utils import run_bass_kernel_spmd

    if "nc" not in _CACHE:
        _CACHE["nc"] = build()
    nc = _CACHE["nc"]

    x = np.ascontiguousarray(np.asarray(x, dtype=np.float32))
    common = {
        "W1": np.ascontiguousarray(np.asarray(W1, dtype=np.float32)),
        "b1": np.ascontiguousarray(np.asarray(b1, dtype=np.float32)),
        "W2": np.ascontiguousarray(np.asarray(W2, dtype=np.float32)),
        "b2": np.ascontiguousarray(np.asarray(b2, dtype=np.float32)),
        "W3": np.ascontiguousarray(np.asarray(W3, dtype=np.float32)),
        "b3": np.ascontiguousarray(np.asarray(b3, dtype=np.float32)),
    }
    shards = np.split(x, NCORES, axis=0)
    in_maps = [{"x": np.ascontiguousarray(shards[c]), **common} for c in range(NCORES)]

    res = run_bass_kernel_spmd(nc, in_maps, core_ids=list(range(NCORES)))
    full = np.concatenate(
        [_unpermute(res.results[c]["out"]) for c in range(NCORES)], axis=1
    )  # [3, NX]
    y = full[0].reshape(NX, 1).astype(np.float32)
    v1 = full[1].reshape(NX, 1).astype(np.float32)
    v2 = full[2].reshape(NX, 1).astype(np.float32)
    return (y, v1, v2)



# revision 12
# speedup vs baseline: 1.0479x; 1.0479x over previous
"""TRN2 Bass kernel for nn_DerivNet2D.

Reference computation (per sample x in R^2):
    h1 = W1 @ x + b1;  z1 = tanh(h1)            (1024)
    h2 = W2 @ z1 + b2; z2 = tanh(h2)            (512)
    y  = W3 @ z2 + b3                           (1)
    dy/dx_k = W3 @ (dz2 * (W2 @ (dz1 * W1[:,k])))   k = 1, 2
    returns (y, v1, v2) = (y, dy/dx2, -dy/dx1)

Strategy (v4):
  * Pure data parallel: x split into 8 shards of 8192 samples; weights
    replicated.  SPMD module via run_bass_kernel_spmd.
  * All weight/input layouts prepared host-side (transposes, bias rows),
    no on-chip preprocessing: PE starts immediately and stays warm.
  * Reverse-mode gradient: A = w3*(1-z2^2); B = W2.T @ A;
    C = (z1^2-1)*B = -dz1*B;  v rows = Wyv.T @ C  (signs fixed on host).
  * L1 with bias folded into the matmul (K=3: x1, x2, ones row), 4
    row-group-packed concurrent MMs.  y and dy/dx share one col-packed
    matmul family: 12 chunk MMs over 4 PE column groups; partial [3,512]
    results summed on the host.
  * Everything f32/f32r (same PE rate as bf16: 1 cycle/row) except q2
    (z2^2) in bf16; per the numpy error budget this keeps rel err ~1e-3.
  * Software pipeline: iteration emits [fwd/bwd/yv of tile T-1][L1 of
    tile T] so the ACT tanh chain of tile T overlaps the PE-heavy phases
    of tile T-1.
  * ACT: q1=z1^2 merged Square + 4x tanh(z2) + 8x tanh(z1).
    DVE: per-chunk q2 TT, A tensor_scalar, fused C scalar_tensor_tensor
    from PSUM, yv-partial copy.
"""

import numpy as np
from contextlib import ExitStack

import concourse.bacc as bacc
import concourse.mybir as mybir
import concourse.tile as tile
from concourse.bass import ds, ts

F32 = mybir.dt.float32
F32R = mybir.dt.float32r
BF16 = mybir.dt.bfloat16
AF = mybir.ActivationFunctionType
ALU = mybir.AluOpType

NCORES = 8
NX = 65536
NXL = NX // NCORES      # 8192 samples per core
NT = 512                # samples per tile
TILES = NXL // NT       # 16

_CACHE = {}


def build():
    nc = bacc.Bacc(None, target_bir_lowering=False)
    XTb = nc.dram_tensor("XTb", [3, NXL], F32, kind="ExternalInput")
    W1Tb = nc.dram_tensor("W1Tb", [3, 1024], F32, kind="ExternalInput")
    W2T = nc.dram_tensor("W2T", [1024, 512], F32, kind="ExternalInput")
    W2N = nc.dram_tensor("W2N", [512, 1024], F32, kind="ExternalInput")
    WYV = nc.dram_tensor("WYV", [12, 128, 3], F32, kind="ExternalInput")
    W3S = nc.dram_tensor("W3S", [128, 8], F32, kind="ExternalInput")  # [-w3 | +w3]
    B2S = nc.dram_tensor("B2S", [128, 4], F32, kind="ExternalInput")
    OUT = nc.dram_tensor("out", [3, NXL], F32, kind="ExternalOutput")

    with ExitStack() as ctx:
        tc = ctx.enter_context(tile.TileContext(nc))
        sg = ctx.enter_context(tc.tile_pool(name="sg", bufs=1))
        pxt = ctx.enter_context(tc.tile_pool(name="pxt", bufs=2))
        pz1 = ctx.enter_context(tc.tile_pool(name="pz1", bufs=2))
        pq1 = ctx.enter_context(tc.tile_pool(name="pq1", bufs=2))
        pz2 = ctx.enter_context(tc.tile_pool(name="pz2", bufs=2))
        pq2 = ctx.enter_context(tc.tile_pool(name="pq2", bufs=2))
        pA = ctx.enter_context(tc.tile_pool(name="pA", bufs=2))
        pC = ctx.enter_context(tc.tile_pool(name="pC", bufs=2))
        pyo = ctx.enter_context(tc.tile_pool(name="pyo", bufs=2))
        ph1 = ctx.enter_context(tc.tile_pool(name="ph1", bufs=2, space="PSUM"))
        ph2 = ctx.enter_context(tc.tile_pool(name="ph2", bufs=2, space="PSUM"))
        pB = ctx.enter_context(tc.tile_pool(name="pB", bufs=2, space="PSUM"))
        pyv = ctx.enter_context(tc.tile_pool(name="pyv", bufs=2, space="PSUM"))

        # ---- preload (pure DMA, no compute) --------------------------
        w1t = sg.tile([99, 1024], F32R)
        nc.sync.dma_start(out=w1t[0:3, :], in_=W1Tb[:, :].bitcast(F32R))
        for g in range(1, 4):
            nc.sync.dma_start(out=w1t[32 * g : 32 * g + 3, :], in_=w1t[0:3, :])

        # fwd lhsT: w2t[p, j, m] = W2[c*128+m, j*128+p] for m-block c
        # split DMAs (c, j-quarter) so fwd chunk 0 isn't gated on 2MB
        w2t = sg.tile([128, 8, 512], F32R)
        for c in range(4):
            for jq in range(4):
                nc.sync.dma_start(
                    out=w2t[:, ds(2 * jq, 2), ds(c * 128, 128)],
                    in_=W2T[ds(jq * 256, 256), ds(c * 128, 128)].rearrange(
                        "(j p) m -> p j m", j=2
                    ).bitcast(F32R),
                )
        # bwd lhsT: w2n[p, c, m] = W2[c*128+p, m]; split by (m-chunk k, c-half)
        w2n = sg.tile([128, 4, 1024], F32R)
        for k in range(8):
            for ch in range(2):
                nc.sync.dma_start(
                    out=w2n[:, ds(2 * ch, 2), ds(k * 128, 128)],
                    in_=W2N[ds(ch * 256, 256), ds(k * 128, 128)].rearrange(
                        "(c p) m -> p c m", c=2
                    ).bitcast(F32R),
                )

        wyv = sg.tile([128, 12, 3], F32R)
        nc.sync.dma_start(out=wyv, in_=WYV[:, :, :].rearrange("k p m -> p k m").bitcast(F32R))

        w3s = sg.tile([128, 8], F32)
        nc.sync.dma_start(out=w3s, in_=W3S[:, :])
        b2t = sg.tile([128, 4], F32)
        nc.sync.dma_start(out=b2t, in_=B2S[:, :])

        # ---- software-pipelined main loop ----------------------------
        state = {}

        def emit_l1(T):
            """L1 of tile T: xt load, 8 row-packed K=3 MMs, tanh -> z1r."""
            sl = ds(T * NT, NT)
            xt = pxt.tile([99, NT], F32R, tag="xt", name="xt")
            for g in range(4):
                nc.sync.dma_start(out=xt[32 * g : 32 * g + 3, :], in_=XTb[:, sl].bitcast(F32R))
            z1r = pz1.tile([128, 8, NT], F32R, tag="z1", name="z1r")
            for c1 in range(8):
                g = c1 % 4
                p1 = ph1.tile([128, NT], F32, tag="h1", name="p1")
                nc.tensor.matmul(
                    p1,
                    w1t[32 * g : 32 * g + 3, ts(c1, 128)],
                    xt[32 * g : 32 * g + 3, :],
                    start=True, stop=True,
                    tile_position=(32 * g, 0),
                )
                nc.scalar.activation(z1r[:, c1, :], p1, AF.Tanh)
            state[T] = z1r

        def emit_rest(T):
            """fwd/bwd/yv of tile T (z1r ready from the previous iter)."""
            sl = ds(T * NT, NT)
            z1r = state.pop(T)

            # q1 = z1^2, one merged ACT op (fills ACT queue head; ready
            # immediately, consumed late by the C stt)
            q1 = pq1.tile([128, 8, NT], F32, tag="q1", name="q1")
            nc.scalar.activation(q1[:, :, :], z1r[:, :, :].bitcast(F32), AF.Square)

            # fwd: h2 = W2 @ z1 + b2; per chunk: tanh -> z2, q2 = z2^2,
            # A = q2*(-w3) + w3
            z2 = pz2.tile([128, 4, NT], F32R, tag="z2", name="z2")
            q2 = pq2.tile([128, 4, NT], BF16, tag="q2", name="q2")
            A = pA.tile([128, 4, NT], F32R, tag="A", name="A")
            for c in range(4):
                p2 = ph2.tile([128, NT], F32, tag="h2", name="p2")
                for j in range(8):
                    nc.tensor.matmul(
                        p2,
                        w2t[:, j, ds(c * 128, 128)],
                        z1r[:, j, :],
                        start=(j == 0), stop=(j == 7),
                    )
                nc.scalar.activation(
                    z2[:, c, :], p2, AF.Tanh, bias=b2t[:, c : c + 1]
                )
                nc.vector.tensor_mul(q2[:, c, :], z2[:, c, :].bitcast(F32), z2[:, c, :].bitcast(F32))
                nc.vector.tensor_scalar(
                    out=A[:, c, :], in0=q2[:, c, :],
                    scalar1=w3s[:, c : c + 1], scalar2=w3s[:, 4 + c : 5 + c],
                    op0=ALU.mult, op1=ALU.add,
                )

            # bwd: B = W2.T @ A;  C = (z1^2 - 1) * B = -dz1*B
            C = pC.tile([128, 8, NT], F32R, tag="C", name="C")
            for i in range(8):
                pb = pB.tile([128, NT], F32, tag="B", name="pb")
                for c in range(4):
                    nc.tensor.matmul(
                        pb,
                        w2n[:, c, ds(i * 128, 128)],
                        A[:, c, :],
                        start=(c == 0), stop=(c == 3),
                    )
                nc.vector.scalar_tensor_tensor(
                    out=C[:, i, :], in0=q1[:, i, :], scalar=1.0, in1=pb,
                    op0=ALU.subtract, op1=ALU.mult,
                )

            # y+v: 12 chunk MMs, one serial accumulation into psum[0:3]
            # (col-group packing is rejected by this toolchain: matmul dst
            # partition base must be 0)
            pyvt = pyv.tile([3, NT], F32, tag="yv", name="pyvt")
            for k in range(12):
                rhs = C[:, k, :] if k < 8 else z2[:, k - 8, :]
                nc.tensor.matmul(
                    pyvt[0:3, :], wyv[:, k, :], rhs,
                    start=(k == 0), stop=(k == 11),
                )
            yvs = pyo.tile([3, NT], F32, tag="yvs", name="yvs")
            nc.vector.tensor_copy(yvs, pyvt)
            nc.sync.dma_start(out=OUT[:, sl], in_=yvs[0:3, :])

        for T in range(TILES + 1):
            if T >= 1:
                emit_rest(T - 1)
            if T < TILES:
                emit_l1(T)

    nc.compile()
    return nc


def prep_inputs(x_shard, W1, b1, W2, b2, W3, b3):
    """Host-side layout prep for one core's shard."""
    f32 = np.float32
    xtb = np.empty((3, NXL), f32)
    xtb[0:2] = x_shard.T
    xtb[2] = 1.0
    w1tb = np.empty((3, 1024), f32)
    w1tb[0:2] = W1.T
    w1tb[2] = b1
    wyv = np.zeros((12, 128, 3), f32)
    for i in range(8):
        blk = W1[i * 128 : (i + 1) * 128]
        wyv[i, :, 0] = blk[:, 1]
        wyv[i, :, 1] = blk[:, 0]
    for c in range(4):
        wyv[8 + c, :, 2] = W3[0, c * 128 : (c + 1) * 128]
    w3s = np.empty((128, 8), f32)
    w3r = W3[0].reshape(4, 128).T  # [p, c]
    w3s[:, 0:4] = -w3r
    w3s[:, 4:8] = w3r
    b2s = np.ascontiguousarray(b2.reshape(4, 128).T)
    return {
        "XTb": np.ascontiguousarray(xtb),
        "W1Tb": np.ascontiguousarray(w1tb),
        "W2T": np.ascontiguousarray(W2.T),
        "W2N": np.ascontiguousarray(W2),
        "WYV": wyv,
        "W3S": np.ascontiguousarray(w3s),
        "B2S": np.ascontiguousarray(b2s.astype(f32)),
    }


def postprocess(o, b3):
    """o: [3, NXL] -> (y, v1, v2) for the shard."""
    v1 = -o[0]
    v2 = o[1]
    y = o[2] + b3[0]
    return y, v1, v2


def kernel(x, W1, b1, W2, b2, W3, b3):
    from concourse.bass_utils import run_bass_kernel_spmd

    if "nc" not in _CACHE:
        _CACHE["nc"] = build()
    nc = _CACHE["nc"]

    x = np.asarray(x, dtype=np.float32)
    W1 = np.asarray(W1, dtype=np.float32)
    b1 = np.asarray(b1, dtype=np.float32)
    W2 = np.asarray(W2, dtype=np.float32)
    b2 = np.asarray(b2, dtype=np.float32)
    W3 = np.asarray(W3, dtype=np.float32)
    b3 = np.asarray(b3, dtype=np.float32)

    shards = np.split(x, NCORES, axis=0)
    in_maps = [
        prep_inputs(shards[c], W1, b1, W2, b2, W3, b3) for c in range(NCORES)
    ]
    _CACHE["in_maps"] = in_maps

    res = run_bass_kernel_spmd(nc, in_maps, core_ids=list(range(NCORES)))
    ys, v1s, v2s = [], [], []
    for c in range(NCORES):
        y, v1, v2 = postprocess(res.results[c]["out"], b3)
        ys.append(y)
        v1s.append(v1)
        v2s.append(v2)
    y = np.concatenate(ys).reshape(NX, 1).astype(np.float32)
    v1 = np.concatenate(v1s).reshape(NX, 1).astype(np.float32)
    v2 = np.concatenate(v2s).reshape(NX, 1).astype(np.float32)
    return (y, v1, v2)


# revision 13
# speedup vs baseline: 1.2184x; 1.1626x over previous
"""TRN2 Bass kernel for nn_DerivNet2D.

Reference computation (per sample x in R^2):
    h1 = W1 @ x + b1;  z1 = tanh(h1)            (1024)
    h2 = W2 @ z1 + b2; z2 = tanh(h2)            (512)
    y  = W3 @ z2 + b3                           (1)
    dy/dx_k = W3 @ (dz2 * (W2 @ (dz1 * W1[:,k])))   k = 1, 2
    returns (y, v1, v2) = (y, dy/dx2, -dy/dx1)

Strategy (v5):
  * Pure data parallel: x split into 8 shards of 8192 samples; weights
    replicated.  SPMD module via run_bass_kernel_spmd.
  * All layouts prepared host-side; no on-chip preprocessing.
  * Reverse-mode gradient: A = w3*(1-z2^2); B = W2.T @ A;
    C = (z1^2-1)*B = -dz1*B;  (y,v) rows = Wyv.T @ [z2-chunks, C-chunks]
    as ONE 12-MM accumulation chain (signs fixed on host).
  * L1 with bias folded into the matmul (K=3: x1, x2, ones row).
  * Everything f32/f32r (same PE rate as bf16) except q2 in bf16.
  * Schedule (per steady-state iteration, PE queue order):
      [fwd(T) 4 chains x 8] [yv-z2(T) x4]
      [bwd(T) chain i -> interleave: L1(T+1) pair after even i,
       yv-C(T) chunk i-1 after chain i] [yv tail] -> copy -> out DMA
    so ACT's tanh chain for tile T+1 runs under bwd(T) and the PE never
    waits on elementwise engines in steady state.
  * PSUM banks: ph1 2 + ph2 3 + pB 2 + pyv 1 = 8.
  * DMA queues: xt + w2t on SP(sync), w2n on ACT(scalar), small preloads
    and output stores on GpSimd so streams never head-block each other.
"""

import numpy as np
from contextlib import ExitStack

import concourse.bacc as bacc
import concourse.mybir as mybir
import concourse.tile as tile
from concourse.bass import ds, ts

F32 = mybir.dt.float32
F32R = mybir.dt.float32r
BF16 = mybir.dt.bfloat16
AF = mybir.ActivationFunctionType
ALU = mybir.AluOpType

NCORES = 8
NX = 65536
NXL = NX // NCORES      # 8192 samples per core
NT = 512                # samples per tile
TILES = NXL // NT       # 16

_CACHE = {}


def build():
    nc = bacc.Bacc(None, target_bir_lowering=False)
    XTb = nc.dram_tensor("XTb", [3, NXL], F32, kind="ExternalInput")
    W1Tb = nc.dram_tensor("W1Tb", [3, 1024], F32, kind="ExternalInput")
    W2T = nc.dram_tensor("W2T", [1024, 512], F32, kind="ExternalInput")
    W2N = nc.dram_tensor("W2N", [512, 1024], F32, kind="ExternalInput")
    WYV = nc.dram_tensor("WYV", [12, 128, 3], F32, kind="ExternalInput")
    W3S = nc.dram_tensor("W3S", [128, 8], F32, kind="ExternalInput")  # [-w3 | +w3]
    B2S = nc.dram_tensor("B2S", [128, 4], F32, kind="ExternalInput")
    OUT = nc.dram_tensor("out", [3, NXL], F32, kind="ExternalOutput")

    with ExitStack() as ctx:
        tc = ctx.enter_context(tile.TileContext(nc))
        sg = ctx.enter_context(tc.tile_pool(name="sg", bufs=1))
        pxt = ctx.enter_context(tc.tile_pool(name="pxt", bufs=2))
        pz1 = ctx.enter_context(tc.tile_pool(name="pz1", bufs=2))
        pq1 = ctx.enter_context(tc.tile_pool(name="pq1", bufs=2))
        pz2 = ctx.enter_context(tc.tile_pool(name="pz2", bufs=2))
        pq2 = ctx.enter_context(tc.tile_pool(name="pq2", bufs=2))
        pA = ctx.enter_context(tc.tile_pool(name="pA", bufs=2))
        pC = ctx.enter_context(tc.tile_pool(name="pC", bufs=2))
        pyo = ctx.enter_context(tc.tile_pool(name="pyo", bufs=2))
        ph1 = ctx.enter_context(tc.tile_pool(name="ph1", bufs=2, space="PSUM"))
        ph2 = ctx.enter_context(tc.tile_pool(name="ph2", bufs=3, space="PSUM"))
        pB = ctx.enter_context(tc.tile_pool(name="pB", bufs=2, space="PSUM"))
        pyv = ctx.enter_context(tc.tile_pool(name="pyv", bufs=1, space="PSUM"))

        # ---- preload (pure DMA, split across the three DGE queues) ---
        w1t = sg.tile([3, 1024], F32R)
        nc.sync.dma_start(out=w1t[0:3, :], in_=W1Tb[:, :].bitcast(F32R))

        wyv = sg.tile([128, 12, 3], F32R)
        nc.gpsimd.dma_start(
            out=wyv, in_=WYV[:, :, :].rearrange("k p m -> p k m").bitcast(F32R)
        )
        w3s = sg.tile([128, 8], F32)
        nc.gpsimd.dma_start(out=w3s, in_=W3S[:, :])
        b2t = sg.tile([128, 4], F32)
        nc.gpsimd.dma_start(out=b2t, in_=B2S[:, :])

        # fwd lhsT: w2t[p, j, m] = W2[c*128+m, j*128+p] for m-block c
        w2t = sg.tile([128, 8, 512], F32R)
        for c in range(4):
            for jq in range(4):
                nc.sync.dma_start(
                    out=w2t[:, ds(2 * jq, 2), ds(c * 128, 128)],
                    in_=W2T[ds(jq * 256, 256), ds(c * 128, 128)]
                    .rearrange("(j p) m -> p j m", j=2)
                    .bitcast(F32R),
                )
        # bwd lhsT: w2n[p, c, m] = W2[c*128+p, m]; split by (m-chunk k, c-half)
        w2n = sg.tile([128, 4, 1024], F32R)
        for k in range(8):
            for ch in range(2):
                nc.scalar.dma_start(
                    out=w2n[:, ds(2 * ch, 2), ds(k * 128, 128)],
                    in_=W2N[ds(ch * 256, 256), ds(k * 128, 128)]
                    .rearrange("(c p) m -> p c m", c=2)
                    .bitcast(F32R),
                )

        # ---- software-pipelined main loop ----------------------------
        state = {}

        def emit_xt(T):
            xt = pxt.tile([3, NT], F32R, tag="xt", name="xt")
            nc.sync.dma_start(
                out=xt[0:3, :], in_=XTb[:, ds(T * NT, NT)].bitcast(F32R)
            )
            state[("xt", T)] = xt

        def emit_l1_pair(T, c0):
            """Two L1 chunk MMs + their tanhs (h1 banks drained by ACT)."""
            xt = state[("xt", T)]
            if c0 == 0:
                state[("z1", T)] = pz1.tile(
                    [128, 8, NT], F32R, tag="z1", name="z1r"
                )
            z1r = state[("z1", T)]
            for c1 in (c0, c0 + 1):
                p1 = ph1.tile([128, NT], F32, tag="h1", name="p1")
                nc.tensor.matmul(
                    p1,
                    w1t[0:3, ts(c1, 128)],
                    xt[0:3, :],
                    start=True, stop=True,
                )
                nc.scalar.activation(z1r[:, c1, :], p1, AF.Tanh)
            if c0 == 6:
                state[T] = state.pop(("z1", T))

        for T in range(TILES + 1):
            if T < TILES:
                emit_xt(T)
            if T == 0:
                for c0 in (0, 2, 4, 6):
                    emit_l1_pair(0, c0)
                continue

            # ---------------- rest of tile T-1 ------------------------
            Tm = T - 1
            sl = ds(Tm * NT, NT)
            z1r = state.pop(Tm)
            state.pop(("xt", Tm), None)

            # q1 = z1^2, one merged ACT op
            q1 = pq1.tile([128, 8, NT], F32, tag="q1", name="q1")
            nc.scalar.activation(q1[:, :, :], z1r[:, :, :].bitcast(F32), AF.Square)

            # fwd: h2 = W2 @ z1 + b2; per chunk: tanh, q2 = z2^2,
            # A = q2*(-w3) + w3
            z2 = pz2.tile([128, 4, NT], F32R, tag="z2", name="z2")
            q2 = pq2.tile([128, 4, NT], BF16, tag="q2", name="q2")
            A = pA.tile([128, 4, NT], F32R, tag="A", name="A")
            for c in range(4):
                p2 = ph2.tile([128, NT], F32, tag="h2", name="p2")
                for j in range(8):
                    nc.tensor.matmul(
                        p2,
                        w2t[:, j, ds(c * 128, 128)],
                        z1r[:, j, :],
                        start=(j == 0), stop=(j == 7),
                    )
                nc.scalar.activation(
                    z2[:, c, :], p2, AF.Tanh, bias=b2t[:, c : c + 1]
                )
                nc.vector.tensor_mul(
                    q2[:, c, :], z2[:, c, :].bitcast(F32), z2[:, c, :].bitcast(F32)
                )
                nc.vector.tensor_scalar(
                    out=A[:, c, :], in0=q2[:, c, :],
                    scalar1=w3s[:, c : c + 1], scalar2=w3s[:, 4 + c : 5 + c],
                    op0=ALU.mult, op1=ALU.add,
                )

            # yv chain start: 4 z2-chunk MMs (chunks 8..11 of Wyv)
            pyvt = pyv.tile([3, NT], F32, tag="yv", name="pyvt")
            for k in range(4):
                nc.tensor.matmul(
                    pyvt[0:3, :], wyv[:, 8 + k, :], z2[:, k, :],
                    start=(k == 0), stop=False, skip_group_check=True,
                )

            # bwd chains with L1(T) pairs and yv-C chunks interleaved
            C = pC.tile([128, 8, NT], F32R, tag="C", name="C")
            for i in range(8):
                pb = pB.tile([128, NT], F32, tag="B", name="pb")
                for c in range(4):
                    nc.tensor.matmul(
                        pb,
                        w2n[:, c, ds(i * 128, 128)],
                        A[:, c, :],
                        start=(c == 0), stop=(c == 3),
                    )
                nc.vector.scalar_tensor_tensor(
                    out=C[:, i, :], in0=q1[:, i, :], scalar=1.0, in1=pb,
                    op0=ALU.subtract, op1=ALU.mult,
                )
                if i % 2 == 0 and T < TILES:
                    emit_l1_pair(T, i)
                if i >= 1:
                    nc.tensor.matmul(
                        pyvt[0:3, :], wyv[:, i - 1, :], C[:, i - 1, :],
                        start=False, stop=False, skip_group_check=True,
                    )
            nc.tensor.matmul(
                pyvt[0:3, :], wyv[:, 7, :], C[:, 7, :],
                start=False, stop=True, skip_group_check=True,
            )

            yvs = pyo.tile([3, NT], F32, tag="yvs", name="yvs")
            nc.vector.tensor_copy(yvs, pyvt)
            nc.gpsimd.dma_start(out=OUT[:, sl], in_=yvs[0:3, :])

    nc.compile()
    return nc


def prep_inputs(x_shard, W1, b1, W2, b2, W3, b3):
    """Host-side layout prep for one core's shard."""
    f32 = np.float32
    xtb = np.empty((3, NXL), f32)
    xtb[0:2] = x_shard.T
    xtb[2] = 1.0
    w1tb = np.empty((3, 1024), f32)
    w1tb[0:2] = W1.T
    w1tb[2] = b1
    wyv = np.zeros((12, 128, 3), f32)
    for i in range(8):
        blk = W1[i * 128 : (i + 1) * 128]
        wyv[i, :, 0] = blk[:, 1]
        wyv[i, :, 1] = blk[:, 0]
    for c in range(4):
        wyv[8 + c, :, 2] = W3[0, c * 128 : (c + 1) * 128]
    w3s = np.empty((128, 8), f32)
    w3r = W3[0].reshape(4, 128).T  # [p, c]
    w3s[:, 0:4] = -w3r
    w3s[:, 4:8] = w3r
    b2s = np.ascontiguousarray(b2.reshape(4, 128).T)
    return {
        "XTb": np.ascontiguousarray(xtb),
        "W1Tb": np.ascontiguousarray(w1tb),
        "W2T": np.ascontiguousarray(W2.T),
        "W2N": np.ascontiguousarray(W2),
        "WYV": wyv,
        "W3S": np.ascontiguousarray(w3s),
        "B2S": np.ascontiguousarray(b2s.astype(f32)),
    }


def postprocess(o, b3):
    """o: [3, NXL] -> (y, v1, v2) for the shard."""
    v1 = -o[0]
    v2 = o[1]
    y = o[2] + b3[0]
    return y, v1, v2


def kernel(x, W1, b1, W2, b2, W3, b3):
    from concourse.bass_utils import run_bass_kernel_spmd

    if "nc" not in _CACHE:
        _CACHE["nc"] = build()
    nc = _CACHE["nc"]

    x = np.asarray(x, dtype=np.float32)
    W1 = np.asarray(W1, dtype=np.float32)
    b1 = np.asarray(b1, dtype=np.float32)
    W2 = np.asarray(W2, dtype=np.float32)
    b2 = np.asarray(b2, dtype=np.float32)
    W3 = np.asarray(W3, dtype=np.float32)
    b3 = np.asarray(b3, dtype=np.float32)

    shards = np.split(x, NCORES, axis=0)
    in_maps = [
        prep_inputs(shards[c], W1, b1, W2, b2, W3, b3) for c in range(NCORES)
    ]
    _CACHE["in_maps"] = in_maps

    res = run_bass_kernel_spmd(nc, in_maps, core_ids=list(range(NCORES)))
    ys, v1s, v2s = [], [], []
    for c in range(NCORES):
        y, v1, v2 = postprocess(res.results[c]["out"], b3)
        ys.append(y)
        v1s.append(v1)
        v2s.append(v2)
    y = np.concatenate(ys).reshape(NX, 1).astype(np.float32)
    v1 = np.concatenate(v1s).reshape(NX, 1).astype(np.float32)
    v2 = np.concatenate(v2s).reshape(NX, 1).astype(np.float32)
    return (y, v1, v2)


# revision 14
# speedup vs baseline: 1.2504x; 1.0263x over previous
"""TRN2 Bass kernel for nn_DerivNet2D.

Reference computation (per sample x in R^2):
    h1 = W1 @ x + b1;  z1 = tanh(h1)            (1024)
    h2 = W2 @ z1 + b2; z2 = tanh(h2)            (512)
    y  = W3 @ z2 + b3                           (1)
    dy/dx_k = W3 @ (dz2 * (W2 @ (dz1 * W1[:,k])))   k = 1, 2
    returns (y, v1, v2) = (y, dy/dx2, -dy/dx1)

Strategy (v5):
  * Pure data parallel: x split into 8 shards of 8192 samples; weights
    replicated.  SPMD module via run_bass_kernel_spmd.
  * All layouts prepared host-side; no on-chip preprocessing.
  * Reverse-mode gradient: A = w3*(1-z2^2); B = W2.T @ A;
    C = (z1^2-1)*B = -dz1*B;  (y,v) rows = Wyv.T @ [z2-chunks, C-chunks]
    as ONE 12-MM accumulation chain (signs fixed on host).
  * L1 with bias folded into the matmul (K=3: x1, x2, ones row).
  * Everything f32/f32r (same PE rate as bf16) except q2 in bf16.
  * Schedule (per steady-state iteration, PE queue order):
      [fwd(T) 4 chains x 8] [yv-z2(T) x4]
      [bwd(T) chain i -> interleave: L1(T+1) pair after even i,
       yv-C(T) chunk i-1 after chain i] [yv tail] -> copy -> out DMA
    so ACT's tanh chain for tile T+1 runs under bwd(T) and the PE never
    waits on elementwise engines in steady state.
  * PSUM banks: ph1 2 + ph2 3 + pB 2 + pyv 1 = 8.
  * DMA queues: xt + w2t on SP(sync), w2n on ACT(scalar), small preloads
    and output stores on GpSimd so streams never head-block each other.
"""

import numpy as np
from contextlib import ExitStack

import concourse.bacc as bacc
import concourse.mybir as mybir
import concourse.tile as tile
from concourse.bass import ds, ts

F32 = mybir.dt.float32
F32R = mybir.dt.float32r
BF16 = mybir.dt.bfloat16
AF = mybir.ActivationFunctionType
ALU = mybir.AluOpType

NCORES = 8
NX = 65536
NXL = NX // NCORES      # 8192 samples per core
NT = 512                # samples per tile
TILES = NXL // NT       # 16

_CACHE = {}


def build():
    nc = bacc.Bacc(None, target_bir_lowering=False)
    XTb = nc.dram_tensor("XTb", [3, NXL], F32, kind="ExternalInput")
    W1Tb = nc.dram_tensor("W1Tb", [3, 1024], F32, kind="ExternalInput")
    W2T = nc.dram_tensor("W2T", [1024, 512], F32, kind="ExternalInput")
    W2N = nc.dram_tensor("W2N", [512, 1024], F32, kind="ExternalInput")
    WYV = nc.dram_tensor("WYV", [12, 128, 3], F32, kind="ExternalInput")
    W3S = nc.dram_tensor("W3S", [128, 8], F32, kind="ExternalInput")  # [-w3 | +w3]
    B2S = nc.dram_tensor("B2S", [128, 4], F32, kind="ExternalInput")
    OUT = nc.dram_tensor("out", [3, NXL], F32, kind="ExternalOutput")

    with ExitStack() as ctx:
        tc = ctx.enter_context(tile.TileContext(nc))
        sg = ctx.enter_context(tc.tile_pool(name="sg", bufs=1))
        pxt = ctx.enter_context(tc.tile_pool(name="pxt", bufs=4))
        pz1 = ctx.enter_context(tc.tile_pool(name="pz1", bufs=2))
        pq1 = ctx.enter_context(tc.tile_pool(name="pq1", bufs=2))
        pz2 = ctx.enter_context(tc.tile_pool(name="pz2", bufs=2))
        pq2 = ctx.enter_context(tc.tile_pool(name="pq2", bufs=2))
        pA = ctx.enter_context(tc.tile_pool(name="pA", bufs=2))
        pC = ctx.enter_context(tc.tile_pool(name="pC", bufs=2))
        pyo = ctx.enter_context(tc.tile_pool(name="pyo", bufs=2))
        ph1 = ctx.enter_context(tc.tile_pool(name="ph1", bufs=2, space="PSUM"))
        ph2 = ctx.enter_context(tc.tile_pool(name="ph2", bufs=3, space="PSUM"))
        pB = ctx.enter_context(tc.tile_pool(name="pB", bufs=2, space="PSUM"))
        pyv = ctx.enter_context(tc.tile_pool(name="pyv", bufs=1, space="PSUM"))

        # ---- preload (pure DMA, split across the three DGE queues) ---
        w1t = sg.tile([35, 1024], F32R)
        nc.sync.dma_start(out=w1t[0:3, :], in_=W1Tb[:, :].bitcast(F32R))
        nc.sync.dma_start(out=w1t[32:35, :], in_=W1Tb[:, :].bitcast(F32R))

        wyv = sg.tile([128, 12, 3], F32R)
        nc.gpsimd.dma_start(
            out=wyv, in_=WYV[:, :, :].rearrange("k p m -> p k m").bitcast(F32R)
        )
        w3s = sg.tile([128, 8], F32)
        nc.gpsimd.dma_start(out=w3s, in_=W3S[:, :])
        b2t = sg.tile([128, 4], F32)
        nc.gpsimd.dma_start(out=b2t, in_=B2S[:, :])

        # ---- software-pipelined main loop ----------------------------
        state = {}

        def emit_xt(T):
            xt = pxt.tile([35, NT], F32R, tag="xt", name="xt")
            nc.sync.dma_start(
                out=xt[0:3, :], in_=XTb[:, ds(T * NT, NT)].bitcast(F32R)
            )
            nc.sync.dma_start(
                out=xt[32:35, :], in_=XTb[:, ds(T * NT, NT)].bitcast(F32R)
            )
            state[("xt", T)] = xt

        for _t in range(min(4, TILES)):
            emit_xt(_t)

        # fwd lhsT: w2t[p, j, m] = W2[c*128+m, j*128+p] for m-block c
        w2t = sg.tile([128, 8, 512], F32R)
        for c in range(4):
            for jq in range(4):
                nc.sync.dma_start(
                    out=w2t[:, ds(2 * jq, 2), ds(c * 128, 128)],
                    in_=W2T[ds(jq * 256, 256), ds(c * 128, 128)]
                    .rearrange("(j p) m -> p j m", j=2)
                    .bitcast(F32R),
                )
        # bwd lhsT: w2n[p, c, m] = W2[c*128+p, m]; split by (m-chunk k, c-half)
        w2n = sg.tile([128, 4, 1024], F32R)
        for k in range(8):
            for ch in range(2):
                nc.scalar.dma_start(
                    out=w2n[:, ds(2 * ch, 2), ds(k * 128, 128)],
                    in_=W2N[ds(ch * 256, 256), ds(k * 128, 128)]
                    .rearrange("(c p) m -> p c m", c=2)
                    .bitcast(F32R),
                )

        def emit_l1_pair(T, c0):
            """Two L1 chunk MMs + their tanhs (h1 banks drained by ACT)."""
            xt = state[("xt", T)]
            if c0 == 0:
                state[("z1", T)] = pz1.tile(
                    [128, 8, NT], F32R, tag="z1", name="z1r"
                )
            z1r = state[("z1", T)]
            for g, c1 in ((0, c0), (32, c0 + 1)):
                p1 = ph1.tile([128, NT], F32, tag="h1", name="p1")
                nc.tensor.matmul(
                    p1,
                    w1t[g : g + 3, ts(c1, 128)],
                    xt[g : g + 3, :],
                    start=True, stop=True,
                    tile_position=(g, 0),
                )
                nc.scalar.activation(z1r[:, c1, :], p1, AF.Tanh)
            if c0 == 6:
                state[T] = state.pop(("z1", T))

        for T in range(TILES + 1):
            if 4 <= T + 3 < TILES:
                emit_xt(T + 3)
            if T == 0:
                for c0 in (0, 2, 4, 6):
                    emit_l1_pair(0, c0)
                continue

            # ---------------- rest of tile T-1 ------------------------
            Tm = T - 1
            sl = ds(Tm * NT, NT)
            z1r = state.pop(Tm)
            state.pop(("xt", Tm), None)

            # q1 = z1^2, one merged ACT op
            q1 = pq1.tile([128, 8, NT], F32, tag="q1", name="q1")
            nc.scalar.activation(q1[:, :, :], z1r[:, :, :].bitcast(F32), AF.Square)

            # fwd: h2 = W2 @ z1 + b2; per chunk: tanh, q2 = z2^2,
            # A = q2*(-w3) + w3
            z2 = pz2.tile([128, 4, NT], F32R, tag="z2", name="z2")
            q2 = pq2.tile([128, 4, NT], BF16, tag="q2", name="q2")
            A = pA.tile([128, 4, NT], F32R, tag="A", name="A")
            for c in range(4):
                p2 = ph2.tile([128, NT], F32, tag="h2", name="p2")
                for j in range(8):
                    nc.tensor.matmul(
                        p2,
                        w2t[:, j, ds(c * 128, 128)],
                        z1r[:, j, :],
                        start=(j == 0), stop=(j == 7),
                    )
                nc.scalar.activation(
                    z2[:, c, :], p2, AF.Tanh, bias=b2t[:, c : c + 1]
                )
                nc.vector.tensor_mul(
                    q2[:, c, :], z2[:, c, :].bitcast(F32), z2[:, c, :].bitcast(F32)
                )
                nc.vector.tensor_scalar(
                    out=A[:, c, :], in0=q2[:, c, :],
                    scalar1=w3s[:, c : c + 1], scalar2=w3s[:, 4 + c : 5 + c],
                    op0=ALU.mult, op1=ALU.add,
                )

            # yv chain start: 4 z2-chunk MMs (chunks 8..11 of Wyv)
            pyvt = pyv.tile([3, NT], F32, tag="yv", name="pyvt")
            for k in range(4):
                nc.tensor.matmul(
                    pyvt[0:3, :], wyv[:, 8 + k, :], z2[:, k, :],
                    start=(k == 0), stop=False, skip_group_check=True,
                )

            # bwd chains with L1(T) pairs and yv-C chunks interleaved
            C = pC.tile([128, 8, NT], F32R, tag="C", name="C")
            for i in range(8):
                pb = pB.tile([128, NT], F32, tag="B", name="pb")
                for c in range(4):
                    nc.tensor.matmul(
                        pb,
                        w2n[:, c, ds(i * 128, 128)],
                        A[:, c, :],
                        start=(c == 0), stop=(c == 3),
                    )
                nc.vector.scalar_tensor_tensor(
                    out=C[:, i, :], in0=q1[:, i, :], scalar=1.0, in1=pb,
                    op0=ALU.subtract, op1=ALU.mult,
                )
                if i % 2 == 0 and T < TILES:
                    emit_l1_pair(T, i)
            for k in range(8):
                nc.tensor.matmul(
                    pyvt[0:3, :], wyv[:, k, :], C[:, k, :],
                    start=False, stop=(k == 7), skip_group_check=True,
                )

            yvs = pyo.tile([3, NT], F32, tag="yvs", name="yvs")
            nc.vector.tensor_copy(yvs, pyvt)
            nc.gpsimd.dma_start(out=OUT[:, sl], in_=yvs[0:3, :])

    nc.compile()
    return nc


def prep_inputs(x_shard, W1, b1, W2, b2, W3, b3):
    """Host-side layout prep for one core's shard."""
    f32 = np.float32
    xtb = np.empty((3, NXL), f32)
    xtb[0:2] = x_shard.T
    xtb[2] = 1.0
    w1tb = np.empty((3, 1024), f32)
    w1tb[0:2] = W1.T
    w1tb[2] = b1
    wyv = np.zeros((12, 128, 3), f32)
    for i in range(8):
        blk = W1[i * 128 : (i + 1) * 128]
        wyv[i, :, 0] = blk[:, 1]
        wyv[i, :, 1] = blk[:, 0]
    for c in range(4):
        wyv[8 + c, :, 2] = W3[0, c * 128 : (c + 1) * 128]
    w3s = np.empty((128, 8), f32)
    w3r = W3[0].reshape(4, 128).T  # [p, c]
    w3s[:, 0:4] = -w3r
    w3s[:, 4:8] = w3r
    b2s = np.ascontiguousarray(b2.reshape(4, 128).T)
    return {
        "XTb": np.ascontiguousarray(xtb),
        "W1Tb": np.ascontiguousarray(w1tb),
        "W2T": np.ascontiguousarray(W2.T),
        "W2N": np.ascontiguousarray(W2),
        "WYV": wyv,
        "W3S": np.ascontiguousarray(w3s),
        "B2S": np.ascontiguousarray(b2s.astype(f32)),
    }


def postprocess(o, b3):
    """o: [3, NXL] -> (y, v1, v2) for the shard."""
    v1 = -o[0]
    v2 = o[1]
    y = o[2] + b3[0]
    return y, v1, v2


def kernel(x, W1, b1, W2, b2, W3, b3):
    from concourse.bass_utils import run_bass_kernel_spmd

    if "nc" not in _CACHE:
        _CACHE["nc"] = build()
    nc = _CACHE["nc"]

    x = np.asarray(x, dtype=np.float32)
    W1 = np.asarray(W1, dtype=np.float32)
    b1 = np.asarray(b1, dtype=np.float32)
    W2 = np.asarray(W2, dtype=np.float32)
    b2 = np.asarray(b2, dtype=np.float32)
    W3 = np.asarray(W3, dtype=np.float32)
    b3 = np.asarray(b3, dtype=np.float32)

    shards = np.split(x, NCORES, axis=0)
    in_maps = [
        prep_inputs(shards[c], W1, b1, W2, b2, W3, b3) for c in range(NCORES)
    ]
    _CACHE["in_maps"] = in_maps

    res = run_bass_kernel_spmd(nc, in_maps, core_ids=list(range(NCORES)))
    ys, v1s, v2s = [], [], []
    for c in range(NCORES):
        y, v1, v2 = postprocess(res.results[c]["out"], b3)
        ys.append(y)
        v1s.append(v1)
        v2s.append(v2)
    y = np.concatenate(ys).reshape(NX, 1).astype(np.float32)
    v1 = np.concatenate(v1s).reshape(NX, 1).astype(np.float32)
    v2 = np.concatenate(v2s).reshape(NX, 1).astype(np.float32)
    return (y, v1, v2)


# revision 15
# speedup vs baseline: 1.2543x; 1.0031x over previous
"""TRN2 Bass kernel for nn_DerivNet2D.

Reference computation (per sample x in R^2):
    h1 = W1 @ x + b1;  z1 = tanh(h1)            (1024)
    h2 = W2 @ z1 + b2; z2 = tanh(h2)            (512)
    y  = W3 @ z2 + b3                           (1)
    dy/dx_k = W3 @ (dz2 * (W2 @ (dz1 * W1[:,k])))   k = 1, 2
    returns (y, v1, v2) = (y, dy/dx2, -dy/dx1)

Strategy (v5):
  * Pure data parallel: x split into 8 shards of 8192 samples; weights
    replicated.  SPMD module via run_bass_kernel_spmd.
  * All layouts prepared host-side; no on-chip preprocessing.
  * Reverse-mode gradient: A = w3*(1-z2^2); B = W2.T @ A;
    C = (z1^2-1)*B = -dz1*B;  (y,v) rows = Wyv.T @ [z2-chunks, C-chunks]
    as ONE 12-MM accumulation chain (signs fixed on host).
  * L1 with bias folded into the matmul (K=3: x1, x2, ones row).
  * Everything f32/f32r (same PE rate as bf16) except q2 in bf16.
  * Schedule (per steady-state iteration, PE queue order):
      [fwd(T) 4 chains x 8] [yv-z2(T) x4]
      [bwd(T) chain i -> interleave: L1(T+1) pair after even i,
       yv-C(T) chunk i-1 after chain i] [yv tail] -> copy -> out DMA
    so ACT's tanh chain for tile T+1 runs under bwd(T) and the PE never
    waits on elementwise engines in steady state.
  * PSUM banks: ph1 2 + ph2 3 + pB 2 + pyv 1 = 8.
  * DMA queues: xt + w2t on SP(sync), w2n on ACT(scalar), small preloads
    and output stores on GpSimd so streams never head-block each other.
"""

import numpy as np
from contextlib import ExitStack

import concourse.bacc as bacc
import concourse.mybir as mybir
import concourse.tile as tile
from concourse.bass import ds, ts

F32 = mybir.dt.float32
F32R = mybir.dt.float32r
BF16 = mybir.dt.bfloat16
AF = mybir.ActivationFunctionType
ALU = mybir.AluOpType

NCORES = 8
NX = 65536
NXL = NX // NCORES      # 8192 samples per core
NT = 512                # samples per tile
TILES = NXL // NT       # 16

_CACHE = {}


def build():
    nc = bacc.Bacc(None, target_bir_lowering=False)
    XTb = nc.dram_tensor("XTb", [8, NXL], BF16, kind="ExternalInput")
    W1Tb = nc.dram_tensor("W1Tb", [8, 1024], BF16, kind="ExternalInput")
    W2T = nc.dram_tensor("W2T", [1024, 512], F32, kind="ExternalInput")
    W2N = nc.dram_tensor("W2N", [512, 1024], F32, kind="ExternalInput")
    WYV = nc.dram_tensor("WYV", [12, 128, 3], F32, kind="ExternalInput")
    W3S = nc.dram_tensor("W3S", [128, 8], F32, kind="ExternalInput")  # [-w3 | +w3]
    B2S = nc.dram_tensor("B2S", [128, 4], F32, kind="ExternalInput")
    OUT = nc.dram_tensor("out", [3, NXL], F32, kind="ExternalOutput")

    with ExitStack() as ctx:
        tc = ctx.enter_context(tile.TileContext(nc))
        sg = ctx.enter_context(tc.tile_pool(name="sg", bufs=1))
        pxt = ctx.enter_context(tc.tile_pool(name="pxt", bufs=4))
        pz1 = ctx.enter_context(tc.tile_pool(name="pz1", bufs=2))
        pq1 = ctx.enter_context(tc.tile_pool(name="pq1", bufs=2))
        pz2 = ctx.enter_context(tc.tile_pool(name="pz2", bufs=2))
        pq2 = ctx.enter_context(tc.tile_pool(name="pq2", bufs=2))
        pA = ctx.enter_context(tc.tile_pool(name="pA", bufs=2))
        pC = ctx.enter_context(tc.tile_pool(name="pC", bufs=2))
        pyo = ctx.enter_context(tc.tile_pool(name="pyo", bufs=2))
        ph1 = ctx.enter_context(tc.tile_pool(name="ph1", bufs=2, space="PSUM"))
        ph2 = ctx.enter_context(tc.tile_pool(name="ph2", bufs=3, space="PSUM"))
        pB = ctx.enter_context(tc.tile_pool(name="pB", bufs=2, space="PSUM"))
        pyv = ctx.enter_context(tc.tile_pool(name="pyv", bufs=1, space="PSUM"))

        # ---- preload (pure DMA, split across the three DGE queues) ---
        w1t = sg.tile([40, 1024], BF16)
        nc.sync.dma_start(out=w1t[0:8, :], in_=W1Tb[:, :])
        nc.sync.dma_start(out=w1t[32:40, :], in_=W1Tb[:, :])

        wyv = sg.tile([128, 12, 3], F32R)
        nc.gpsimd.dma_start(
            out=wyv, in_=WYV[:, :, :].rearrange("k p m -> p k m").bitcast(F32R)
        )
        w3s = sg.tile([128, 8], F32)
        nc.gpsimd.dma_start(out=w3s, in_=W3S[:, :])
        b2t = sg.tile([128, 4], F32)
        nc.gpsimd.dma_start(out=b2t, in_=B2S[:, :])

        # ---- software-pipelined main loop ----------------------------
        state = {}

        def emit_xt(T):
            xt = pxt.tile([40, NT], BF16, tag="xt", name="xt")
            nc.sync.dma_start(out=xt[0:8, :], in_=XTb[:, ds(T * NT, NT)])
            nc.sync.dma_start(out=xt[32:40, :], in_=XTb[:, ds(T * NT, NT)])
            state[("xt", T)] = xt

        for _t in range(min(4, TILES)):
            emit_xt(_t)

        # fwd lhsT: w2t[p, j, m] = W2[c*128+m, j*128+p] for m-block c
        w2t = sg.tile([128, 8, 512], F32R)
        for c in range(4):
            for jq in range(4):
                nc.sync.dma_start(
                    out=w2t[:, ds(2 * jq, 2), ds(c * 128, 128)],
                    in_=W2T[ds(jq * 256, 256), ds(c * 128, 128)]
                    .rearrange("(j p) m -> p j m", j=2)
                    .bitcast(F32R),
                )
        # bwd lhsT: w2n[p, c, m] = W2[c*128+p, m]; split by (m-chunk k, c-half)
        w2n = sg.tile([128, 4, 1024], F32R)
        for k in range(8):
            for ch in range(2):
                nc.scalar.dma_start(
                    out=w2n[:, ds(2 * ch, 2), ds(k * 128, 128)],
                    in_=W2N[ds(ch * 256, 256), ds(k * 128, 128)]
                    .rearrange("(c p) m -> p c m", c=2)
                    .bitcast(F32R),
                )

        def emit_l1_pair(T, c0):
            """Two L1 chunk MMs + their tanhs (h1 banks drained by ACT)."""
            xt = state[("xt", T)]
            if c0 == 0:
                state[("z1", T)] = pz1.tile(
                    [128, 8, NT], F32R, tag="z1", name="z1r"
                )
            z1r = state[("z1", T)]
            for g, c1 in ((0, c0), (32, c0 + 1)):
                p1 = ph1.tile([128, NT], F32, tag="h1", name="p1")
                nc.tensor.matmul(
                    p1,
                    w1t[g : g + 8, ts(c1, 128)],
                    xt[g : g + 8, :],
                    start=True, stop=True,
                    tile_position=(g, 0),
                )
                nc.scalar.activation(z1r[:, c1, :], p1, AF.Tanh)
            if c0 == 6:
                state[T] = state.pop(("z1", T))

        for T in range(TILES + 1):
            if 4 <= T + 3 < TILES:
                emit_xt(T + 3)
            if T == 0:
                for c0 in (0, 2, 4, 6):
                    emit_l1_pair(0, c0)
                continue

            # ---------------- rest of tile T-1 ------------------------
            Tm = T - 1
            sl = ds(Tm * NT, NT)
            z1r = state.pop(Tm)
            state.pop(("xt", Tm), None)

            # q1 = z1^2 on the (otherwise idle) GpSimd engine, two halves
            q1 = pq1.tile([128, 8, NT], F32, tag="q1", name="q1")
            for h in range(2):
                nc.gpsimd.tensor_mul(
                    q1[:, ds(4 * h, 4), :],
                    z1r[:, ds(4 * h, 4), :].bitcast(F32),
                    z1r[:, ds(4 * h, 4), :].bitcast(F32),
                )

            # fwd: h2 = W2 @ z1 + b2; per chunk: tanh, q2 = z2^2,
            # A = q2*(-w3) + w3
            z2 = pz2.tile([128, 4, NT], F32R, tag="z2", name="z2")
            q2 = pq2.tile([128, 4, NT], BF16, tag="q2", name="q2")
            A = pA.tile([128, 4, NT], F32R, tag="A", name="A")
            for c in range(4):
                p2 = ph2.tile([128, NT], F32, tag="h2", name="p2")
                for j in range(8):
                    nc.tensor.matmul(
                        p2,
                        w2t[:, j, ds(c * 128, 128)],
                        z1r[:, j, :],
                        start=(j == 0), stop=(j == 7),
                    )
                nc.scalar.activation(
                    z2[:, c, :], p2, AF.Tanh, bias=b2t[:, c : c + 1]
                )
                nc.vector.tensor_mul(
                    q2[:, c, :], z2[:, c, :].bitcast(F32), z2[:, c, :].bitcast(F32)
                )
                nc.scalar.activation(
                    A[:, c, :], q2[:, c, :], AF.Identity,
                    bias=w3s[:, 4 + c : 5 + c], scale=w3s[:, c : c + 1],
                )

            # yv chain start: 4 z2-chunk MMs (chunks 8..11 of Wyv)
            pyvt = pyv.tile([3, NT], F32, tag="yv", name="pyvt")
            for k in range(4):
                nc.tensor.matmul(
                    pyvt[0:3, :], wyv[:, 8 + k, :], z2[:, k, :],
                    start=(k == 0), stop=False, skip_group_check=True,
                )

            # bwd chains with L1(T) pairs and yv-C chunks interleaved
            C = pC.tile([128, 8, NT], F32R, tag="C", name="C")
            for i in range(8):
                pb = pB.tile([128, NT], F32, tag="B", name="pb")
                for c in range(4):
                    nc.tensor.matmul(
                        pb,
                        w2n[:, c, ds(i * 128, 128)],
                        A[:, c, :],
                        start=(c == 0), stop=(c == 3),
                    )
                nc.vector.scalar_tensor_tensor(
                    out=C[:, i, :], in0=q1[:, i, :], scalar=1.0, in1=pb,
                    op0=ALU.subtract, op1=ALU.mult,
                )
                if i % 2 == 0 and T < TILES:
                    emit_l1_pair(T, i)
            for k in range(8):
                nc.tensor.matmul(
                    pyvt[0:3, :], wyv[:, k, :], C[:, k, :],
                    start=False, stop=(k == 7), skip_group_check=True,
                )

            yvs = pyo.tile([3, NT], F32, tag="yvs", name="yvs")
            nc.vector.tensor_copy(yvs, pyvt)
            nc.gpsimd.dma_start(out=OUT[:, sl], in_=yvs[0:3, :])

    nc.compile()
    return nc


def prep_inputs(x_shard, W1, b1, W2, b2, W3, b3):
    """Host-side layout prep for one core's shard."""
    import ml_dtypes

    f32 = np.float32
    bf16 = ml_dtypes.bfloat16
    # L1 in bf16 with hi/lo splitting: K=8 rows
    #   lhsT: [w1a_hi, w1a_hi, w1a_lo, w1b_hi, w1b_hi, w1b_lo, b1_hi, b1_lo]
    #   rhs:  [x1hi,   x1lo,   x1hi,   x2hi,   x2lo,   x2hi,   1,     1   ]
    x1 = x_shard[:, 0].astype(f32)
    x2 = x_shard[:, 1].astype(f32)
    x1hi = x1.astype(bf16)
    x1lo = (x1 - x1hi.astype(f32)).astype(bf16)
    x2hi = x2.astype(bf16)
    x2lo = (x2 - x2hi.astype(f32)).astype(bf16)
    one = np.ones(NXL, bf16)
    xtb = np.stack([x1hi, x1lo, x1hi, x2hi, x2lo, x2hi, one, one])
    w1a = W1[:, 0].astype(f32)
    w1b = W1[:, 1].astype(f32)
    w1a_hi = w1a.astype(bf16)
    w1a_lo = (w1a - w1a_hi.astype(f32)).astype(bf16)
    w1b_hi = w1b.astype(bf16)
    w1b_lo = (w1b - w1b_hi.astype(f32)).astype(bf16)
    b1hi = b1.astype(bf16)
    b1lo = (b1.astype(f32) - b1hi.astype(f32)).astype(bf16)
    w1tb = np.stack([w1a_hi, w1a_hi, w1a_lo, w1b_hi, w1b_hi, w1b_lo, b1hi, b1lo])
    wyv = np.zeros((12, 128, 3), f32)
    for i in range(8):
        blk = W1[i * 128 : (i + 1) * 128]
        wyv[i, :, 0] = blk[:, 1]
        wyv[i, :, 1] = blk[:, 0]
    for c in range(4):
        wyv[8 + c, :, 2] = W3[0, c * 128 : (c + 1) * 128]
    w3s = np.empty((128, 8), f32)
    w3r = W3[0].reshape(4, 128).T  # [p, c]
    w3s[:, 0:4] = -w3r
    w3s[:, 4:8] = w3r
    b2s = np.ascontiguousarray(b2.reshape(4, 128).T)
    return {
        "XTb": np.ascontiguousarray(xtb),
        "W1Tb": np.ascontiguousarray(w1tb),
        "W2T": np.ascontiguousarray(W2.T),
        "W2N": np.ascontiguousarray(W2),
        "WYV": wyv,
        "W3S": np.ascontiguousarray(w3s),
        "B2S": np.ascontiguousarray(b2s.astype(f32)),
    }


def postprocess(o, b3):
    """o: [3, NXL] -> (y, v1, v2) for the shard."""
    v1 = -o[0]
    v2 = o[1]
    y = o[2] + b3[0]
    return y, v1, v2


def kernel(x, W1, b1, W2, b2, W3, b3):
    from concourse.bass_utils import run_bass_kernel_spmd

    if "nc" not in _CACHE:
        _CACHE["nc"] = build()
    nc = _CACHE["nc"]

    x = np.asarray(x, dtype=np.float32)
    W1 = np.asarray(W1, dtype=np.float32)
    b1 = np.asarray(b1, dtype=np.float32)
    W2 = np.asarray(W2, dtype=np.float32)
    b2 = np.asarray(b2, dtype=np.float32)
    W3 = np.asarray(W3, dtype=np.float32)
    b3 = np.asarray(b3, dtype=np.float32)

    shards = np.split(x, NCORES, axis=0)
    in_maps = [
        prep_inputs(shards[c], W1, b1, W2, b2, W3, b3) for c in range(NCORES)
    ]
    _CACHE["in_maps"] = in_maps

    res = run_bass_kernel_spmd(nc, in_maps, core_ids=list(range(NCORES)))
    ys, v1s, v2s = [], [], []
    for c in range(NCORES):
        y, v1, v2 = postprocess(res.results[c]["out"], b3)
        ys.append(y)
        v1s.append(v1)
        v2s.append(v2)
    y = np.concatenate(ys).reshape(NX, 1).astype(np.float32)
    v1 = np.concatenate(v1s).reshape(NX, 1).astype(np.float32)
    v2 = np.concatenate(v2s).reshape(NX, 1).astype(np.float32)
    return (y, v1, v2)


# revision 18
# speedup vs baseline: 1.2783x; 1.0191x over previous
"""TRN2 Bass kernel for nn_DerivNet2D.

Reference computation (per sample x in R^2):
    h1 = W1 @ x + b1;  z1 = tanh(h1)            (1024)
    h2 = W2 @ z1 + b2; z2 = tanh(h2)            (512)
    y  = W3 @ z2 + b3                           (1)
    dy/dx_k = W3 @ (dz2 * (W2 @ (dz1 * W1[:,k])))   k = 1, 2
    returns (y, v1, v2) = (y, dy/dx2, -dy/dx1)

Strategy (v5):
  * Pure data parallel: x split into 8 shards of 8192 samples; weights
    replicated.  SPMD module via run_bass_kernel_spmd.
  * All layouts prepared host-side; no on-chip preprocessing.
  * Reverse-mode gradient: A = w3*(1-z2^2); B = W2.T @ A;
    C = (z1^2-1)*B = -dz1*B;  (y,v) rows = Wyv.T @ [z2-chunks, C-chunks]
    as ONE 12-MM accumulation chain (signs fixed on host).
  * L1 with bias folded into the matmul (K=3: x1, x2, ones row).
  * Everything f32/f32r (same PE rate as bf16) except q2 in bf16.
  * Schedule (per steady-state iteration, PE queue order):
      [fwd(T) 4 chains x 8] [yv-z2(T) x4]
      [bwd(T) chain i -> interleave: L1(T+1) pair after even i,
       yv-C(T) chunk i-1 after chain i] [yv tail] -> copy -> out DMA
    so ACT's tanh chain for tile T+1 runs under bwd(T) and the PE never
    waits on elementwise engines in steady state.
  * PSUM banks: ph1 2 + ph2 3 + pB 2 + pyv 1 = 8.
  * DMA queues: xt + w2t on SP(sync), w2n on ACT(scalar), small preloads
    and output stores on GpSimd so streams never head-block each other.
"""

import numpy as np
from contextlib import ExitStack

import concourse.bacc as bacc
import concourse.mybir as mybir
import concourse.tile as tile
from concourse.bass import ds, ts

F32 = mybir.dt.float32
F32R = mybir.dt.float32r
BF16 = mybir.dt.bfloat16
AF = mybir.ActivationFunctionType
ALU = mybir.AluOpType

NCORES = 8
NX = 65536
NXL = NX // NCORES      # 8192 samples per core
NT = 512                # samples per tile
TILES = NXL // NT       # 16

_CACHE = {}


def build():
    nc = bacc.Bacc(None, target_bir_lowering=False)
    XTb = nc.dram_tensor("XTb", [8, NXL], BF16, kind="ExternalInput")
    W1Tb = nc.dram_tensor("W1Tb", [8, 1024], BF16, kind="ExternalInput")
    W2T = nc.dram_tensor("W2T", [1024, 512], F32, kind="ExternalInput")
    W2N = nc.dram_tensor("W2N", [512, 1024], F32, kind="ExternalInput")
    WYV = nc.dram_tensor("WYV", [12, 128, 3], F32, kind="ExternalInput")
    W3S = nc.dram_tensor("W3S", [128, 8], F32, kind="ExternalInput")  # [-w3 | +w3]
    B2S = nc.dram_tensor("B2S", [128, 4], F32, kind="ExternalInput")
    OUT = nc.dram_tensor("out", [3, NXL], F32, kind="ExternalOutput")

    with ExitStack() as ctx:
        tc = ctx.enter_context(tile.TileContext(nc))
        sg = ctx.enter_context(tc.tile_pool(name="sg", bufs=1))
        pxt = ctx.enter_context(tc.tile_pool(name="pxt", bufs=4))
        pz1 = ctx.enter_context(tc.tile_pool(name="pz1", bufs=2))
        pq1 = ctx.enter_context(tc.tile_pool(name="pq1", bufs=2))
        pz2 = ctx.enter_context(tc.tile_pool(name="pz2", bufs=2))
        pq2 = ctx.enter_context(tc.tile_pool(name="pq2", bufs=2))
        pA = ctx.enter_context(tc.tile_pool(name="pA", bufs=2))
        pC = ctx.enter_context(tc.tile_pool(name="pC", bufs=2))
        pyo = ctx.enter_context(tc.tile_pool(name="pyo", bufs=2))
        ph1 = ctx.enter_context(tc.tile_pool(name="ph1", bufs=2, space="PSUM"))
        ph2 = ctx.enter_context(tc.tile_pool(name="ph2", bufs=2, space="PSUM"))
        pB = ctx.enter_context(tc.tile_pool(name="pB", bufs=3, space="PSUM"))
        pyv = ctx.enter_context(tc.tile_pool(name="pyv", bufs=1, space="PSUM"))

        # ---- PE warmup: ~24 dummy MMs on a zeroed tile so the HAM
        # clock-gate reaches 2.4 GHz while the weight DMAs stream in ----
        warm = sg.tile([128, NT], BF16)
        nc.vector.memset(warm, 0.0)
        for _ in range(24):
            pw = ph1.tile([128, NT], F32, tag="h1", name="p1")
            nc.tensor.matmul(pw, warm[:, 0:128], warm, start=True, stop=True)

        # ---- preload (pure DMA, split across the three DGE queues) ---
        w1t = sg.tile([40, 1024], BF16)
        nc.sync.dma_start(out=w1t[0:8, :], in_=W1Tb[:, :])
        nc.sync.dma_start(out=w1t[32:40, :], in_=W1Tb[:, :])

        wyv = sg.tile([128, 12, 3], F32R)
        nc.gpsimd.dma_start(
            out=wyv, in_=WYV[:, :, :].rearrange("k p m -> p k m").bitcast(F32R)
        )
        w3s = sg.tile([128, 8], F32)
        nc.gpsimd.dma_start(out=w3s, in_=W3S[:, :])
        b2t = sg.tile([128, 4], F32)
        nc.gpsimd.dma_start(out=b2t, in_=B2S[:, :])

        # ---- software-pipelined main loop ----------------------------
        state = {}

        def emit_xt(T):
            xt = pxt.tile([40, NT], BF16, tag="xt", name="xt")
            nc.sync.dma_start(out=xt[0:8, :], in_=XTb[:, ds(T * NT, NT)])
            nc.sync.dma_start(out=xt[32:40, :], in_=XTb[:, ds(T * NT, NT)])
            state[("xt", T)] = xt

        for _t in range(min(4, TILES)):
            emit_xt(_t)

        # fwd lhsT: w2t[p, j, m] = W2[c*128+m, j*128+p] for m-block c
        w2t = sg.tile([128, 8, 512], F32R)
        for c in range(4):
            for jq in range(4):
                nc.sync.dma_start(
                    out=w2t[:, ds(2 * jq, 2), ds(c * 128, 128)],
                    in_=W2T[ds(jq * 256, 256), ds(c * 128, 128)]
                    .rearrange("(j p) m -> p j m", j=2)
                    .bitcast(F32R),
                )
        # bwd lhsT: w2n[p, c, m] = W2[c*128+p, m]; split by (m-chunk k, c-half)
        w2n = sg.tile([128, 4, 1024], F32R)
        for k in range(8):
            for ch in range(2):
                nc.scalar.dma_start(
                    out=w2n[:, ds(2 * ch, 2), ds(k * 128, 128)],
                    in_=W2N[ds(ch * 256, 256), ds(k * 128, 128)]
                    .rearrange("(c p) m -> p c m", c=2)
                    .bitcast(F32R),
                )

        def emit_l1_pair(T, c0):
            """Two L1 chunk MMs + their tanhs (h1 banks drained by ACT)."""
            xt = state[("xt", T)]
            if c0 == 0:
                state[("z1", T)] = pz1.tile(
                    [128, 8, NT], F32R, tag="z1", name="z1r"
                )
            z1r = state[("z1", T)]
            for g, c1 in ((0, c0), (32, c0 + 1)):
                p1 = ph1.tile([128, NT], F32, tag="h1", name="p1")
                nc.tensor.matmul(
                    p1,
                    w1t[g : g + 8, ts(c1, 128)],
                    xt[g : g + 8, :],
                    start=True, stop=True,
                    tile_position=(g, 0),
                )
                nc.scalar.activation(z1r[:, c1, :], p1, AF.Tanh)
            if c0 == 6:
                state[T] = state.pop(("z1", T))

        for T in range(TILES + 1):
            if 4 <= T + 3 < TILES:
                emit_xt(T + 3)
            if T == 0:
                for c0 in (0, 2, 4, 6):
                    emit_l1_pair(0, c0)
                continue

            # ---------------- rest of tile T-1 ------------------------
            Tm = T - 1
            sl = ds(Tm * NT, NT)
            z1r = state.pop(Tm)
            state.pop(("xt", Tm), None)

            # q1 = z1^2 on the (otherwise idle) GpSimd engine, two halves
            q1 = pq1.tile([128, 8, NT], F32, tag="q1", name="q1")
            for h in range(2):
                nc.gpsimd.tensor_mul(
                    q1[:, ds(4 * h, 4), :],
                    z1r[:, ds(4 * h, 4), :].bitcast(F32),
                    z1r[:, ds(4 * h, 4), :].bitcast(F32),
                )

            # fwd: h2 = W2 @ z1 + b2; per chunk: tanh, q2 = z2^2,
            # A = q2*(-w3) + w3
            z2 = pz2.tile([128, 4, NT], F32R, tag="z2", name="z2")
            q2 = pq2.tile([128, 4, NT], BF16, tag="q2", name="q2")
            A = pA.tile([128, 4, NT], F32R, tag="A", name="A")
            for c in range(4):
                p2 = ph2.tile([128, NT], F32, tag="h2", name="p2")
                for j in range(8):
                    nc.tensor.matmul(
                        p2,
                        w2t[:, j, ds(c * 128, 128)],
                        z1r[:, j, :],
                        start=(j == 0), stop=(j == 7),
                    )
                nc.scalar.activation(
                    z2[:, c, :], p2, AF.Tanh, bias=b2t[:, c : c + 1]
                )
                nc.vector.tensor_mul(
                    q2[:, c, :], z2[:, c, :].bitcast(F32), z2[:, c, :].bitcast(F32)
                )
                nc.scalar.activation(
                    A[:, c, :], q2[:, c, :], AF.Identity,
                    bias=w3s[:, 4 + c : 5 + c], scale=w3s[:, c : c + 1],
                )

            # yv chain start: 4 z2-chunk MMs (chunks 8..11 of Wyv)
            pyvt = pyv.tile([3, NT], F32, tag="yv", name="pyvt")
            for k in range(4):
                nc.tensor.matmul(
                    pyvt[0:3, :], wyv[:, 8 + k, :], z2[:, k, :],
                    start=(k == 0), stop=False, skip_group_check=True,
                )

            # bwd chains with L1(T) pairs and yv-C chunks interleaved
            C = pC.tile([128, 8, NT], F32R, tag="C", name="C")
            for i in range(8):
                pb = pB.tile([128, NT], F32, tag="B", name="pb")
                for c in range(4):
                    nc.tensor.matmul(
                        pb,
                        w2n[:, c, ds(i * 128, 128)],
                        A[:, c, :],
                        start=(c == 0), stop=(c == 3),
                    )
                nc.vector.scalar_tensor_tensor(
                    out=C[:, i, :], in0=q1[:, i, :], scalar=1.0, in1=pb,
                    op0=ALU.subtract, op1=ALU.mult,
                )
                if i % 2 == 0 and T < TILES:
                    emit_l1_pair(T, i)
            for k in range(8):
                nc.tensor.matmul(
                    pyvt[0:3, :], wyv[:, k, :], C[:, k, :],
                    start=False, stop=(k == 7), skip_group_check=True,
                )

            yvs = pyo.tile([3, NT], F32, tag="yvs", name="yvs")
            nc.vector.tensor_copy(yvs, pyvt)
            nc.gpsimd.dma_start(out=OUT[:, sl], in_=yvs[0:3, :])

    nc.compile()
    return nc


def prep_inputs(x_shard, W1, b1, W2, b2, W3, b3):
    """Host-side layout prep for one core's shard."""
    import ml_dtypes

    f32 = np.float32
    bf16 = ml_dtypes.bfloat16
    # L1 in bf16 with hi/lo splitting: K=8 rows
    #   lhsT: [w1a_hi, w1a_hi, w1a_lo, w1b_hi, w1b_hi, w1b_lo, b1_hi, b1_lo]
    #   rhs:  [x1hi,   x1lo,   x1hi,   x2hi,   x2lo,   x2hi,   1,     1   ]
    x1 = x_shard[:, 0].astype(f32)
    x2 = x_shard[:, 1].astype(f32)
    x1hi = x1.astype(bf16)
    x1lo = (x1 - x1hi.astype(f32)).astype(bf16)
    x2hi = x2.astype(bf16)
    x2lo = (x2 - x2hi.astype(f32)).astype(bf16)
    one = np.ones(NXL, bf16)
    xtb = np.stack([x1hi, x1lo, x1hi, x2hi, x2lo, x2hi, one, one])
    w1a = W1[:, 0].astype(f32)
    w1b = W1[:, 1].astype(f32)
    w1a_hi = w1a.astype(bf16)
    w1a_lo = (w1a - w1a_hi.astype(f32)).astype(bf16)
    w1b_hi = w1b.astype(bf16)
    w1b_lo = (w1b - w1b_hi.astype(f32)).astype(bf16)
    b1hi = b1.astype(bf16)
    b1lo = (b1.astype(f32) - b1hi.astype(f32)).astype(bf16)
    w1tb = np.stack([w1a_hi, w1a_hi, w1a_lo, w1b_hi, w1b_hi, w1b_lo, b1hi, b1lo])
    wyv = np.zeros((12, 128, 3), f32)
    for i in range(8):
        blk = W1[i * 128 : (i + 1) * 128]
        wyv[i, :, 0] = blk[:, 1]
        wyv[i, :, 1] = blk[:, 0]
    for c in range(4):
        wyv[8 + c, :, 2] = W3[0, c * 128 : (c + 1) * 128]
    w3s = np.empty((128, 8), f32)
    w3r = W3[0].reshape(4, 128).T  # [p, c]
    w3s[:, 0:4] = -w3r
    w3s[:, 4:8] = w3r
    b2s = np.ascontiguousarray(b2.reshape(4, 128).T)
    return {
        "XTb": np.ascontiguousarray(xtb),
        "W1Tb": np.ascontiguousarray(w1tb),
        "W2T": np.ascontiguousarray(W2.T),
        "W2N": np.ascontiguousarray(W2),
        "WYV": wyv,
        "W3S": np.ascontiguousarray(w3s),
        "B2S": np.ascontiguousarray(b2s.astype(f32)),
    }


def postprocess(o, b3):
    """o: [3, NXL] -> (y, v1, v2) for the shard."""
    v1 = -o[0]
    v2 = o[1]
    y = o[2] + b3[0]
    return y, v1, v2


def kernel(x, W1, b1, W2, b2, W3, b3):
    from concourse.bass_utils import run_bass_kernel_spmd

    if "nc" not in _CACHE:
        _CACHE["nc"] = build()
    nc = _CACHE["nc"]

    x = np.asarray(x, dtype=np.float32)
    W1 = np.asarray(W1, dtype=np.float32)
    b1 = np.asarray(b1, dtype=np.float32)
    W2 = np.asarray(W2, dtype=np.float32)
    b2 = np.asarray(b2, dtype=np.float32)
    W3 = np.asarray(W3, dtype=np.float32)
    b3 = np.asarray(b3, dtype=np.float32)

    shards = np.split(x, NCORES, axis=0)
    in_maps = [
        prep_inputs(shards[c], W1, b1, W2, b2, W3, b3) for c in range(NCORES)
    ]
    _CACHE["in_maps"] = in_maps

    res = run_bass_kernel_spmd(nc, in_maps, core_ids=list(range(NCORES)))
    ys, v1s, v2s = [], [], []
    for c in range(NCORES):
        y, v1, v2 = postprocess(res.results[c]["out"], b3)
        ys.append(y)
        v1s.append(v1)
        v2s.append(v2)
    y = np.concatenate(ys).reshape(NX, 1).astype(np.float32)
    v1 = np.concatenate(v1s).reshape(NX, 1).astype(np.float32)
    v2 = np.concatenate(v2s).reshape(NX, 1).astype(np.float32)
    return (y, v1, v2)
